# revision 6
# baseline (speedup 1.0000x reference)
"""VQ codebook encoding (nn_Encoding) kernel for 8 Trainium2 NeuronCores.

Reference computation (per batch b):
    xf = x[b].reshape(C, N).T                     # (N, C), N = H*W
    s_nk = scale_k * (||x_n||^2 - 2 x_n.c_k + ||c_k||^2)
    aw = softmax_k(s)
    enc[b] = aw^T xf - (sum_n aw)_k c_k           # (K, C)

Distribution: data-parallel over batch B across the 8 cores (2 batches per
core), codewords/scale replicated.

v3 design (per batch, per core):
  - host: xh = bf16(x) (the only big stream, 2B/elem); x2_n = ||x_n||^2
    computed exactly on host in f64 and streamed as a tiny (128, 72) f32
    tile; softmax offset m folded into bias (exact softmax is invariant
    to a per-pixel-constant offset).
  - mm1 (PE):  T_nk = sum_c xh * W1, W1 = -2*scale_k*c_k (bf16),
               accumulated chunk-wide into Tc [128, 9*32] PSUM.
  - transpose: DMA XBAR transpose (SBUF->SBUF, one instr per [128,1152]
               c-slab -> [n128, ti, c128]), issued on the ACT/DVE DGE
               queues so it runs parallel to the HBM loads on sync.
               No PE transpose, no PSUM evacuation.
  - softmax (chunk-wide, 9 tiles at once to amortize per-instr overheads):
      z = scale_k*x2 + bias_k + Tc   (2 tensor ops + 1 STT on [128,288])
      e = exp(z) on ACT (one [128,288] instr), d = segmented reduce (DVE),
      aw = e * (1/d) bf16 (DVE, broadcast AP)
  - mm2 (PE):  enc[k, c] += sum_n aw * xT   -- aw STATIONARY (32-col load),
               xT moving (512 streams); awsum via ones column (1 stream).
               Issued one chunk behind (pend queue) so the PE never waits
               on the cross-engine softmax latency.
  - tail:      enc += awsum_k * (-c_kc) (one STT on 32 partitions), DMA out.
"""

import os

os.environ.setdefault("JAX_PLATFORMS", "")

import numpy as np
import ml_dtypes
from contextlib import ExitStack

import concourse.bacc as bacc
import concourse.bass as bass
import concourse.mybir as mybir
import concourse.tile as tile
from concourse.bass_utils import run_bass_kernel_spmd

bf16 = ml_dtypes.bfloat16
F32 = mybir.dt.float32
BF = mybir.dt.bfloat16

B, C, H, W = 16, 512, 96, 96
N = H * W            # 9216
K = 32
NCORES = 8
BPC = B // NCORES    # batches per core = 2
NCH = 8              # N chunks per batch
NC = N // NCH        # 1152 pixels per chunk
NT = NC // 128       # 9 tiles per chunk
CCH = C // 128       # 4 contraction chunks
NTILES = N // 128    # 72 tiles per batch

_mult = mybir.AluOpType.mult
_add = mybir.AluOpType.add

_compiled = {}


def _build_program(reps=1):
    nc = bacc.Bacc("TRN2", target_bir_lowering=False, debug=False,
                   num_devices=NCORES)

    xh_d = nc.dram_tensor("xh", [BPC, CCH, 128, N], BF, kind="ExternalInput").ap()
    x2_d = nc.dram_tensor("x2p", [BPC, 128, NTILES], F32, kind="ExternalInput").ap()
    w1t_d = nc.dram_tensor("w1t", [128, CCH, K], BF, kind="ExternalInput").ap()
    scaleb_d = nc.dram_tensor("scaleb", [128, K], F32, kind="ExternalInput").ap()
    biasb_d = nc.dram_tensor("biasb", [128, K], F32, kind="ExternalInput").ap()
    cwneg_d = nc.dram_tensor("cwneg", [K, C], F32, kind="ExternalInput").ap()
    onescol_d = nc.dram_tensor("ones_col", [128, 1], BF, kind="ExternalInput").ap()
    out_d = nc.dram_tensor("enc", [BPC, K, C], F32, kind="ExternalOutput").ap()

    with tile.TileContext(nc) as tc, ExitStack() as ctx:
        const = ctx.enter_context(tc.tile_pool(name="const", bufs=1))
        xpool = ctx.enter_context(tc.tile_pool(name="xh", bufs=2))
        xTpool = ctx.enter_context(tc.tile_pool(name="xT", bufs=3))
        x2pool = ctx.enter_context(tc.tile_pool(name="x2", bufs=2))
        psT = ctx.enter_context(tc.tile_pool(name="psT", bufs=2, space="PSUM"))
        psE = ctx.enter_context(tc.tile_pool(name="psE", bufs=1, space="PSUM"))
        psA = ctx.enter_context(tc.tile_pool(name="psA", bufs=1, space="PSUM"))
        sbZ = ctx.enter_context(tc.tile_pool(name="sbZ", bufs=4))
        sbE = ctx.enter_context(tc.tile_pool(name="sbE", bufs=2))
        sbD = ctx.enter_context(tc.tile_pool(name="sbD", bufs=4))
        sbAw = ctx.enter_context(tc.tile_pool(name="sbAw", bufs=3))
        sbOut = ctx.enter_context(tc.tile_pool(name="sbOut", bufs=2))

        w1t = const.tile([128, CCH, K], BF)
        nc.sync.dma_start(w1t[:], w1t_d)
        scaleb = const.tile([128, K], F32)
        nc.sync.dma_start(scaleb[:], scaleb_d)
        biasb = const.tile([128, K], F32)
        nc.sync.dma_start(biasb[:], biasb_d)
        cwneg = const.tile([K, C], F32)
        nc.sync.dma_start(cwneg[:], cwneg_d)
        onescol = const.tile([128, 1], BF)
        nc.sync.dma_start(onescol[:], onescol_d)

        loop_cm = tc.For_i(0, reps, 1) if reps > 1 else None
        if loop_cm is not None:
            ctx.enter_context(loop_cm)

        for b in range(BPC):
            encB = psE.tile([K, C], F32)       # (k, c) accumulated over N
            awsumP = psA.tile([K, 1], F32)

            x2sb = x2pool.tile([128, NTILES], F32)
            nc.sync.dma_start(x2sb[:], x2_d[b])

            pend = []                          # (gi, xT_slice, aw_slice)

            def issue_mm2(ent):
                gi_, xT_, aw_ = ent
                first = gi_ == 0
                last = gi_ == NTILES - 1
                nc.tensor.matmul(encB[:], aw_, xT_,
                                 start=first, stop=last,
                                 skip_group_check=True)
                nc.tensor.matmul(awsumP[:], aw_, onescol[:],
                                 start=first, stop=last,
                                 skip_group_check=True)

            for ch in range(NCH):
                xh_t = xpool.tile([128, CCH, NC], BF)
                nc.sync.dma_start(
                    xh_t[:],
                    xh_d[b, :, :, ch * NC:(ch + 1) * NC].rearrange("c p n -> p c n"))

                # DMA XBAR transposes: [c128, n1152] -> [n128, ti, c128],
                # spread across the ACT and DVE DGE queues.
                xTc = xTpool.tile([128, NT, CCH, 128], BF)
                for ci in range(CCH):
                    nc.scalar.dma_start(xTc[:, :, ci, :], xh_t[:, ci, :],
                                        transpose=True)

                Tc = psT.tile([128, NT, K], F32, tag="T")
                for ti in range(NT):
                    for ci in range(CCH):
                        a = xh_t[:, ci, bass.ts(ti, 128)]
                        nc.tensor.matmul(Tc[:, ti, :], a, w1t[:, ci, :],
                                         start=(ci == 0), stop=(ci == CCH - 1),
                                         skip_group_check=True)
                    if pend:
                        issue_mm2(pend.pop(0))

                # chunk-wide softmax over [128, NT, K]
                x2_bc = x2sb[:, ch * NT:(ch + 1) * NT].unsqueeze(2) \
                    .broadcast_to((128, NT, K))
                scale_bc = scaleb[:].unsqueeze(1).broadcast_to((128, NT, K))
                bias_bc = biasb[:].unsqueeze(1).broadcast_to((128, NT, K))

                z1 = sbZ.tile([128, NT, K], F32, tag="z1")
                nc.vector.tensor_mul(z1[:], scale_bc, x2_bc)
                z2 = sbZ.tile([128, NT, K], F32, tag="z2")
                nc.vector.tensor_add(z2[:], z1[:], bias_bc)
                z = sbZ.tile([128, NT, K], F32, tag="z")
                nc.vector.scalar_tensor_tensor(
                    z[:], Tc[:], 1.0, z2[:], op0=_mult, op1=_add)

                e = sbE.tile([128, NT, K], F32)
                nc.scalar.activation(e[:], z[:],
                                     mybir.ActivationFunctionType.Exp)
                d9 = sbD.tile([128, NT], F32, tag="d")
                nc.vector.tensor_reduce(d9[:], e[:],
                                        axis=mybir.AxisListType.X, op=_add)
                dinv9 = sbD.tile([128, NT], F32, tag="dinv")
                nc.vector.reciprocal(dinv9[:], d9[:])
                awc = sbAw.tile([128, NT, K], BF)
                dinv_bc = dinv9[:].unsqueeze(2).broadcast_to((128, NT, K))
                nc.vector.tensor_mul(awc[:], e[:], dinv_bc)

                for ti in range(NT):
                    gi = ch * NT + ti
                    pend.append((gi, xTc[:, ti], awc[:, ti, :]))

            for ent in pend:
                issue_mm2(ent)
            pend = []

            # batch tail: enc = encB + awsum * (-c)
            awsum_sb = sbD.tile([K, 1], F32, tag="awsum")
            nc.scalar.copy(awsum_sb[:], awsumP[:])
            encOut = sbOut.tile([K, C], F32, tag="encOut")
            nc.vector.scalar_tensor_tensor(
                encOut[:], cwneg[:], awsum_sb[:], encB[:],
                op0=_mult, op1=_add)
            nc.sync.dma_start(out_d[b], encOut[:])

    nc.finalize()
    return nc


def _prep_inputs(x, codewords, scale):
    xf = np.ascontiguousarray(x.reshape(B, C, N))
    xh = xf.astype(bf16)
    xh4 = xh.reshape(B, CCH, 128, N)

    cw64 = codewords.astype(np.float64)
    sc64 = scale.astype(np.float64)
    alpha = float(sc64.max())
    # Constant softmax offset m ~ alpha * x2: exact softmax is invariant to
    # any per-pixel-constant offset; it only has to keep exp() in range.
    x2flat = np.einsum('bcn,bcn->bn', xf.astype(np.float64), xf.astype(np.float64))
    x2lo, x2hi = float(x2flat.min()), float(x2flat.max())
    m = alpha * 0.5 * (x2lo + x2hi)
    spread = abs(alpha) * 0.5 * (x2hi - x2lo) + 10.0
    assert spread < 60.0, (
        f"constant-offset softmax unsafe: |max_k s - m| can reach {spread:.1f}"
    )
    c2 = (cw64 ** 2).sum(1)
    bias = (sc64 * c2 - m).astype(np.float32)
    w1 = (-2.0 * sc64[:, None] * cw64).astype(bf16)        # (K, C)
    w1t = np.ascontiguousarray(
        w1.T.reshape(CCH, 128, K).transpose(1, 0, 2))       # (128, CCH, K)
    scaleb = np.broadcast_to(scale.astype(np.float32), (128, K)).copy()
    biasb = np.broadcast_to(bias, (128, K)).copy()
    cwneg = np.ascontiguousarray(-codewords.astype(np.float32))  # (K, C)

    # x2 tiled as (B, 128, NTILES): x2p[b, p, g] = x2[b, 128 g + p]
    x2p = np.ascontiguousarray(
        x2flat.astype(np.float32).reshape(B, NTILES, 128).transpose(0, 2, 1))

    consts = {
        "w1t": w1t,
        "biasb": biasb,
        "scaleb": scaleb,
        "cwneg": cwneg,
        "ones_col": np.ones((128, 1), bf16),
    }
    in_maps = []
    for core in range(NCORES):
        m_ = dict(consts)
        m_["xh"] = xh4[core * BPC:(core + 1) * BPC]
        m_["x2p"] = x2p[core * BPC:(core + 1) * BPC]
        in_maps.append(m_)
    return in_maps


def kernel(x, codewords, scale, _trace=False, _return_results=False, _reps=1):
    key = ("prog", _reps)
    if key not in _compiled:
        _compiled[key] = _build_program(reps=_reps)
    nc = _compiled[key]
    in_maps = _prep_inputs(np.asarray(x), np.asarray(codewords),
                           np.asarray(scale))
    res = run_bass_kernel_spmd(nc, in_maps, list(range(NCORES)), trace=_trace)
    out = np.empty((B, K, C), np.float32)
    for core in range(NCORES):
        o = res.results[core]["enc"]                        # (BPC, K, C)
        for b in range(BPC):
            out[core * BPC + b] = o[b]
    if _return_results:
        return out, res
    return out


# revision 8
# speedup vs baseline: 1.6691x; 1.6691x over previous
"""VQ codebook encoding (nn_Encoding) kernel for 8 Trainium2 NeuronCores.

Reference computation (per batch b):
    xf = x[b].reshape(C, N).T                     # (N, C), N = H*W
    s_nk = scale_k * (||x_n||^2 - 2 x_n.c_k + ||c_k||^2)
    aw = softmax_k(s)
    enc[b] = aw^T xf - (sum_n aw)_k c_k           # (K, C)

Distribution: data-parallel over batch B across the 8 cores (2 batches per
core), codewords/scale replicated.

v2.5 design (per batch, per core):
  - host: xh = bf16(x) (the only big stream, 2B/elem); x2_n = ||x_n||^2
    computed exactly on host in f64 and streamed as a tiny (128, 72) f32
    tile; softmax offset m folded into bias (exact softmax is invariant
    to a per-pixel-constant offset).
  - mm1 (PE):  T_nk = sum_c xh * W1, W1 = -2*scale_k*c_k (bf16),
               a-tile stationary (shared with the transpose below);
               accumulated chunk-wide into Tc [128, 9*32] PSUM.
  - transpose: xh tiles transposed on PE (transpose-mode, bf16 PSUM out),
               evacuated to SBUF round-robin on DVE/ACT/Pool.
  - softmax (chunk-wide, 9 tiles at once to amortize per-instr overheads):
      z = scale_k*x2 + bias_k + Tc   (2 tensor ops + 1 STT on [128,288])
      e = exp(z) on ACT (one [128,288] instr), d = segmented reduce (DVE),
      aw = e * (1/d) bf16 (DVE, broadcast AP)
  - mm2 (PE):  enc[k, c] += sum_n aw * xT   -- aw STATIONARY (32-col load),
               xT moving (512 streams); awsum via ones column (1 stream).
               Issued one chunk behind (pend queue) so the PE never waits
               on the cross-engine softmax latency.
  - tail:      enc += awsum_k * (-c_kc) (one STT on 32 partitions), DMA out.
"""

import os

os.environ.setdefault("JAX_PLATFORMS", "")

import numpy as np
import ml_dtypes
from contextlib import ExitStack

import concourse.bacc as bacc
import concourse.bass as bass
import concourse.mybir as mybir
import concourse.tile as tile
from concourse.bass_utils import run_bass_kernel_spmd

bf16 = ml_dtypes.bfloat16
F32 = mybir.dt.float32
BF = mybir.dt.bfloat16

B, C, H, W = 16, 512, 96, 96
N = H * W            # 9216
K = 32
NCORES = 8
BPC = B // NCORES    # batches per core = 2
NCH = 8              # N chunks per batch
NC = N // NCH        # 1152 pixels per chunk
NT = NC // 128       # 9 tiles per chunk
CCH = C // 128       # 4 contraction chunks
NTILES = N // 128    # 72 tiles per batch

_mult = mybir.AluOpType.mult
_add = mybir.AluOpType.add

_compiled = {}


def _build_program(reps=1):
    nc = bacc.Bacc("TRN2", target_bir_lowering=False, debug=False,
                   num_devices=NCORES)

    xh_d = nc.dram_tensor("xh", [BPC, CCH, 128, N], BF, kind="ExternalInput").ap()
    x2_d = nc.dram_tensor("x2p", [BPC, 128, NTILES], F32, kind="ExternalInput").ap()
    w1t_d = nc.dram_tensor("w1t", [128, CCH, K], BF, kind="ExternalInput").ap()
    scaleb_d = nc.dram_tensor("scaleb", [128, K], F32, kind="ExternalInput").ap()
    biasb_d = nc.dram_tensor("biasb", [128, K], F32, kind="ExternalInput").ap()
    cwneg_d = nc.dram_tensor("cwneg", [K, C], F32, kind="ExternalInput").ap()
    ident_d = nc.dram_tensor("ident", [128, 128], BF, kind="ExternalInput").ap()
    onescol_d = nc.dram_tensor("ones_col", [128, 1], BF, kind="ExternalInput").ap()
    out_d = nc.dram_tensor("enc", [BPC, K, C], F32, kind="ExternalOutput").ap()

    with tile.TileContext(nc) as tc, ExitStack() as ctx:
        const = ctx.enter_context(tc.tile_pool(name="const", bufs=1))
        xpool = ctx.enter_context(tc.tile_pool(name="xh", bufs=2))
        x2pool = ctx.enter_context(tc.tile_pool(name="x2", bufs=2))
        psT = ctx.enter_context(tc.tile_pool(name="psT", bufs=2, space="PSUM"))
        psX = ctx.enter_context(tc.tile_pool(name="psX", bufs=3, space="PSUM"))
        psE = ctx.enter_context(tc.tile_pool(name="psE", bufs=1, space="PSUM"))
        psA = ctx.enter_context(tc.tile_pool(name="psA", bufs=1, space="PSUM"))
        sbX = ctx.enter_context(tc.tile_pool(name="sbX", bufs=NT + 4))
        sbZ = ctx.enter_context(tc.tile_pool(name="sbZ", bufs=4))
        sbE = ctx.enter_context(tc.tile_pool(name="sbE", bufs=2))
        sbD = ctx.enter_context(tc.tile_pool(name="sbD", bufs=4))
        sbAw = ctx.enter_context(tc.tile_pool(name="sbAw", bufs=3))
        sbOut = ctx.enter_context(tc.tile_pool(name="sbOut", bufs=2))

        w1t = const.tile([128, CCH, K], BF)
        nc.sync.dma_start(w1t[:], w1t_d)
        scaleb = const.tile([128, K], F32)
        nc.sync.dma_start(scaleb[:], scaleb_d)
        biasb = const.tile([128, K], F32)
        nc.sync.dma_start(biasb[:], biasb_d)
        cwneg = const.tile([K, C], F32)
        nc.sync.dma_start(cwneg[:], cwneg_d)
        ident = const.tile([128, 128], BF)
        nc.sync.dma_start(ident[:], ident_d)
        onescol = const.tile([128, 1], BF)
        nc.sync.dma_start(onescol[:], onescol_d)

        loop_cm = tc.For_i(0, reps, 1) if reps > 1 else None
        if loop_cm is not None:
            ctx.enter_context(loop_cm)

        for b in range(BPC):
            encB = psE.tile([K, C], F32)       # (k, c) accumulated over N
            awsumP = psA.tile([K, 1], F32)

            x2sb = x2pool.tile([128, NTILES], F32)
            nc.sync.dma_start(x2sb[:], x2_d[b])

            pend = []                          # (gi, xT, aw_slice) for mm2

            def issue_mm2(ent):
                gi_, xT_, aw_ = ent
                first = gi_ == 0
                last = gi_ == NTILES - 1
                nc.tensor.matmul(encB[:], aw_, xT_[:],
                                 start=first, stop=last,
                                 skip_group_check=True)
                nc.tensor.matmul(awsumP[:], aw_, onescol[:],
                                 start=first, stop=last,
                                 skip_group_check=True)

            for ch in range(NCH):
                xh_t = xpool.tile([128, CCH, NC], BF)
                nc.sync.dma_start(
                    xh_t[:],
                    xh_d[b, :, :, ch * NC:(ch + 1) * NC].rearrange("c p n -> p c n"))

                Tc = psT.tile([128, NT, K], F32, tag="T")
                xTs = []
                for ti in range(NT):
                    gi = ch * NT + ti
                    Xp = psX.tile([128, C], BF)
                    for ci in range(CCH):
                        a = xh_t[:, ci, bass.ts(ti, 128)]
                        # same stationary operand for both -> weight reuse
                        nc.tensor.matmul(Tc[:, ti, :], a, w1t[:, ci, :],
                                         start=(ci == 0), stop=(ci == CCH - 1),
                                         skip_group_check=True)
                        nc.tensor.transpose(Xp[:, bass.ts(ci, 128)], a, ident[:])

                    xT = sbX.tile([128, C], BF)
                    if gi % 3 == 1:
                        nc.scalar.copy(xT[:], Xp[:])
                    else:
                        nc.vector.tensor_copy(xT[:], Xp[:])
                    xTs.append(xT)

                    if pend:
                        issue_mm2(pend.pop(0))

                # chunk-wide softmax over [128, NT, K]
                x2_bc = x2sb[:, ch * NT:(ch + 1) * NT].unsqueeze(2) \
                    .broadcast_to((128, NT, K))
                scale_bc = scaleb[:].unsqueeze(1).broadcast_to((128, NT, K))
                bias_bc = biasb[:].unsqueeze(1).broadcast_to((128, NT, K))

                z1 = sbZ.tile([128, NT, K], F32, tag="z1")
                nc.vector.tensor_mul(z1[:], scale_bc, x2_bc)
                z2 = sbZ.tile([128, NT, K], F32, tag="z2")
                nc.vector.tensor_add(z2[:], z1[:], bias_bc)
                z = sbZ.tile([128, NT, K], F32, tag="z")
                nc.vector.scalar_tensor_tensor(
                    z[:], Tc[:], 1.0, z2[:], op0=_mult, op1=_add)

                e = sbE.tile([128, NT, K], F32)
                nc.scalar.activation(e[:], z[:],
                                     mybir.ActivationFunctionType.Exp)
                d9 = sbD.tile([128, NT], F32, tag="d")
                nc.vector.tensor_reduce(d9[:], e[:],
                                        axis=mybir.AxisListType.X, op=_add)
                dinv9 = sbD.tile([128, NT], F32, tag="dinv")
                nc.vector.reciprocal(dinv9[:], d9[:])
                awc = sbAw.tile([128, NT, K], BF)
                dinv_bc = dinv9[:].unsqueeze(2).broadcast_to((128, NT, K))
                nc.vector.tensor_mul(awc[:], e[:], dinv_bc)

                for ti in range(NT):
                    gi = ch * NT + ti
                    pend.append((gi, xTs[ti], awc[:, ti, :]))

            for ent in pend:
                issue_mm2(ent)
            pend = []

            # batch tail: enc = encB + awsum * (-c)
            awsum_sb = sbD.tile([K, 1], F32, tag="awsum")
            nc.scalar.copy(awsum_sb[:], awsumP[:])
            encOut = sbOut.tile([K, C], F32, tag="encOut")
            nc.vector.scalar_tensor_tensor(
                encOut[:], cwneg[:], awsum_sb[:], encB[:],
                op0=_mult, op1=_add)
            nc.sync.dma_start(out_d[b], encOut[:])

    nc.finalize()
    return nc


def _prep_inputs(x, codewords, scale):
    xf = np.ascontiguousarray(x.reshape(B, C, N))
    xh = xf.astype(bf16)
    xh4 = xh.reshape(B, CCH, 128, N)

    cw64 = codewords.astype(np.float64)
    sc64 = scale.astype(np.float64)
    alpha = float(sc64.max())
    # Constant softmax offset m ~ alpha * x2: exact softmax is invariant to
    # any per-pixel-constant offset; it only has to keep exp() in range.
    x2flat = np.einsum('bcn,bcn->bn', xf.astype(np.float64), xf.astype(np.float64))
    x2lo, x2hi = float(x2flat.min()), float(x2flat.max())
    m = alpha * 0.5 * (x2lo + x2hi)
    spread = abs(alpha) * 0.5 * (x2hi - x2lo) + 10.0
    assert spread < 60.0, (
        f"constant-offset softmax unsafe: |max_k s - m| can reach {spread:.1f}"
    )
    c2 = (cw64 ** 2).sum(1)
    bias = (sc64 * c2 - m).astype(np.float32)
    w1 = (-2.0 * sc64[:, None] * cw64).astype(bf16)        # (K, C)
    w1t = np.ascontiguousarray(
        w1.T.reshape(CCH, 128, K).transpose(1, 0, 2))       # (128, CCH, K)
    scaleb = np.broadcast_to(scale.astype(np.float32), (128, K)).copy()
    biasb = np.broadcast_to(bias, (128, K)).copy()
    cwneg = np.ascontiguousarray(-codewords.astype(np.float32))  # (K, C)

    # x2 tiled as (B, 128, NTILES): x2p[b, p, g] = x2[b, 128 g + p]
    x2p = np.ascontiguousarray(
        x2flat.astype(np.float32).reshape(B, NTILES, 128).transpose(0, 2, 1))

    consts = {
        "w1t": w1t,
        "biasb": biasb,
        "scaleb": scaleb,
        "cwneg": cwneg,
        "ident": np.eye(128, dtype=bf16),
        "ones_col": np.ones((128, 1), bf16),
    }
    in_maps = []
    for core in range(NCORES):
        m_ = dict(consts)
        m_["xh"] = xh4[core * BPC:(core + 1) * BPC]
        m_["x2p"] = x2p[core * BPC:(core + 1) * BPC]
        in_maps.append(m_)
    return in_maps


def kernel(x, codewords, scale, _trace=False, _return_results=False, _reps=1):
    key = ("prog", _reps)
    if key not in _compiled:
        _compiled[key] = _build_program(reps=_reps)
    nc = _compiled[key]
    in_maps = _prep_inputs(np.asarray(x), np.asarray(codewords),
                           np.asarray(scale))
    res = run_bass_kernel_spmd(nc, in_maps, list(range(NCORES)), trace=_trace)
    out = np.empty((B, K, C), np.float32)
    for core in range(NCORES):
        o = res.results[core]["enc"]                        # (BPC, K, C)
        for b in range(BPC):
            out[core * BPC + b] = o[b]
    if _return_results:
        return out, res
    return out


# revision 26
# speedup vs baseline: 2.2286x; 1.3352x over previous
"""VQ codebook encoding (nn_Encoding) kernel for 8 Trainium2 NeuronCores.

Reference computation (per batch b):
    xf = x[b].reshape(C, N).T                     # (N, C), N = H*W
    s_nk = scale_k * (||x_n||^2 - 2 x_n.c_k + ||c_k||^2)
    aw = softmax_k(s)
    enc[b] = aw^T xf - (sum_n aw)_k c_k           # (K, C)

Distribution: data-parallel over batch B across the 8 cores (2 batches per
core), codewords/scale replicated.

v2.5 design (per batch, per core):
  - host: xh = bf16(x) (the only big stream, 2B/elem); x2_n = ||x_n||^2
    computed exactly on host in f64 and streamed as a tiny (128, 72) f32
    tile; softmax offset m folded into bias (exact softmax is invariant
    to a per-pixel-constant offset).
  - mm1 (PE):  T_nk = sum_c xh * W1, W1 = -2*scale_k*c_k (bf16),
               a-tile stationary (shared with the transpose below);
               accumulated chunk-wide into Tc [128, 9*32] PSUM.
  - transpose: xh tiles transposed on PE (transpose-mode, bf16 PSUM out),
               evacuated to SBUF round-robin on DVE/ACT/Pool.
  - softmax (chunk-wide, 9 tiles at once to amortize per-instr overheads):
      z = scale_k*x2 + bias_k + Tc   (2 tensor ops + 1 STT on [128,288])
      e = exp(z) on ACT (one [128,288] instr), d = segmented reduce (DVE),
      aw = e * (1/d) bf16 (DVE, broadcast AP)
  - mm2 (PE):  enc[k, c] += sum_n aw * xT   -- aw STATIONARY (32-col load),
               xT moving (512 streams); awsum via ones column (1 stream).
               Issued one chunk behind (pend queue) so the PE never waits
               on the cross-engine softmax latency.
  - tail:      enc += awsum_k * (-c_kc) (one STT on 32 partitions), DMA out.
"""

import os

os.environ.setdefault("JAX_PLATFORMS", "")

import numpy as np
import ml_dtypes
from contextlib import ExitStack

import concourse.bacc as bacc
import concourse.bass as bass
import concourse.mybir as mybir
import concourse.tile as tile
from concourse.bass_utils import run_bass_kernel_spmd

bf16 = ml_dtypes.bfloat16
F32 = mybir.dt.float32
BF = mybir.dt.bfloat16

B, C, H, W = 16, 512, 96, 96
N = H * W            # 9216
K = 32
NCORES = 8
BPC = B // NCORES    # batches per core = 2
NCH = 8              # N chunks per batch
NC = N // NCH        # 1152 pixels per chunk
NT = NC // 128       # 9 tiles per chunk
CCH = C // 128       # 4 contraction chunks
NTILES = N // 128    # 72 tiles per batch

_mult = mybir.AluOpType.mult
_add = mybir.AluOpType.add

_compiled = {}


def _build_program(reps=1, stage=5, evac=0, lag=1, awred=1):
    # stage: 0=DMA only, 1=+mm1, 2=+transpose, 3=+evac, 4=+softmax, 5=full
    # evac: 0 = inline [DVE,ACT,DVE]; 1 = all DVE inline;
    #       2 = DVE inline 2/3 + ACT deferred post-softmax 1/3
    # lag: chunks of mm2 backlog kept pending (1 or 2)
    # awred: 0 = per-tile PE awsum matmuls; 1 = DVE chunk reduce + one matmul
    nc = bacc.Bacc("TRN2", target_bir_lowering=False, debug=False,
                   num_devices=NCORES)

    xh_d = nc.dram_tensor("xh", [BPC, CCH, 128, N], BF, kind="ExternalInput").ap()
    x2_d = nc.dram_tensor("x2p", [BPC, 128, NTILES], F32, kind="ExternalInput").ap()
    w1t_d = nc.dram_tensor("w1t", [128, CCH, K], BF, kind="ExternalInput").ap()
    scaleb_d = nc.dram_tensor("scaleb", [128, K], F32, kind="ExternalInput").ap()
    biasb_d = nc.dram_tensor("biasb", [128, K], F32, kind="ExternalInput").ap()
    cwneg_d = nc.dram_tensor("cwneg", [K, C], F32, kind="ExternalInput").ap()
    ident_d = nc.dram_tensor("ident", [128, 128], BF, kind="ExternalInput").ap()
    onescol_d = nc.dram_tensor("ones_col", [128, 1], BF, kind="ExternalInput").ap()
    onescolf_d = nc.dram_tensor("ones_col_f", [128, 1], F32, kind="ExternalInput").ap()
    out_d = nc.dram_tensor("enc", [BPC, K, C], F32, kind="ExternalOutput").ap()

    with tile.TileContext(nc) as tc, ExitStack() as ctx:
        const = ctx.enter_context(tc.tile_pool(name="const", bufs=1))
        xpool = ctx.enter_context(tc.tile_pool(name="xh", bufs=2))
        x2pool = ctx.enter_context(tc.tile_pool(name="x2", bufs=2))
        psT = ctx.enter_context(tc.tile_pool(name="psT", bufs=2, space="PSUM"))
        psX = ctx.enter_context(tc.tile_pool(name="psX", bufs=4, space="PSUM"))
        psE = ctx.enter_context(tc.tile_pool(name="psE", bufs=1, space="PSUM"))
        psA = ctx.enter_context(tc.tile_pool(name="psA", bufs=1, space="PSUM"))
        sbX = ctx.enter_context(tc.tile_pool(name="sbX", bufs=2 * NT + 4))
        sbZ = ctx.enter_context(tc.tile_pool(name="sbZ", bufs=4))
        sbE = ctx.enter_context(tc.tile_pool(name="sbE", bufs=2))
        sbD = ctx.enter_context(tc.tile_pool(name="sbD", bufs=4))
        sbAw = ctx.enter_context(tc.tile_pool(name="sbAw", bufs=4))
        sbOut = ctx.enter_context(tc.tile_pool(name="sbOut", bufs=2))

        w1t = const.tile([128, CCH, K], BF)
        nc.sync.dma_start(w1t[:], w1t_d)
        scaleb = const.tile([128, K], F32)
        nc.sync.dma_start(scaleb[:], scaleb_d)
        biasb = const.tile([128, K], F32)
        nc.sync.dma_start(biasb[:], biasb_d)
        cwneg = const.tile([K, C], F32)
        nc.sync.dma_start(cwneg[:], cwneg_d)
        ident = const.tile([128, 128], BF)
        nc.sync.dma_start(ident[:], ident_d)
        onescol = const.tile([128, 1], BF)
        nc.sync.dma_start(onescol[:], onescol_d)
        onescolf = const.tile([128, 1], F32)
        nc.sync.dma_start(onescolf[:], onescolf_d)

        loop_cm = tc.For_i(0, reps, 1) if reps > 1 else None
        if loop_cm is not None:
            ctx.enter_context(loop_cm)

        for b in range(BPC):
            encB = psE.tile([K, C], F32)       # (k, c) accumulated over N
            awsumP = psA.tile([K, 1], F32)

            x2sb = x2pool.tile([128, NTILES], F32)
            nc.sync.dma_start(x2sb[:], x2_d[b])

            pend = []                          # (gi, xT, aw_slice) for mm2
            deferred = []                      # (Xp, xT) ACT evacs post-softmax

            def issue_mm2(ent):
                gi_, xT_, aw_ = ent
                first = gi_ == 0
                last = gi_ == NTILES - 1
                nc.tensor.matmul(encB[:], aw_, xT_[:],
                                 start=first, stop=last,
                                 skip_group_check=True)
                if awred == 0:
                    nc.tensor.matmul(awsumP[:], aw_, onescol[:],
                                     start=first, stop=last,
                                     skip_group_check=True)

            for ch in range(NCH):
                xh_t = xpool.tile([128, CCH, NC], BF)
                nc.sync.dma_start(
                    xh_t[:],
                    xh_d[b, :, :, ch * NC:(ch + 1) * NC].rearrange("c p n -> p c n"))

                Tc = psT.tile([128, NT, K], F32, tag="T")
                xTs = []
                for ti in range(NT):
                    gi = ch * NT + ti
                    Xp = psX.tile([128, C], BF)
                    for ci in range(CCH):
                        a = xh_t[:, ci, bass.ts(ti, 128)]
                        if stage >= 1:
                            nc.tensor.matmul(Tc[:, ti, :], a, w1t[:, ci, :],
                                             start=(ci == 0), stop=(ci == CCH - 1),
                                             skip_group_check=True)
                        if stage >= 2:
                            nc.tensor.transpose(Xp[:, bass.ts(ci, 128)], a, ident[:])

                    if stage < 3:
                        continue
                    xT = sbX.tile([128, C], BF)
                    if evac == 1 or gi % 3 != 1:
                        nc.vector.tensor_copy(xT[:], Xp[:])
                    elif evac == 0:
                        nc.scalar.copy(xT[:], Xp[:])
                    else:
                        deferred.append((Xp, xT))
                    xTs.append(xT)

                    if stage >= 5 and len(pend) > (lag - 1) * NT:
                        issue_mm2(pend.pop(0))

                if stage < 4:
                    continue
                # chunk-wide softmax over [128, NT, K]
                x2_bc = x2sb[:, ch * NT:(ch + 1) * NT].unsqueeze(2) \
                    .broadcast_to((128, NT, K))
                scale_bc = scaleb[:].unsqueeze(1).broadcast_to((128, NT, K))
                bias_bc = biasb[:].unsqueeze(1).broadcast_to((128, NT, K))

                z1 = sbZ.tile([128, NT, K], F32, tag="z1")
                nc.vector.tensor_mul(z1[:], scale_bc, x2_bc)
                z2 = sbZ.tile([128, NT, K], F32, tag="z2")
                nc.vector.tensor_add(z2[:], z1[:], bias_bc)
                z = sbZ.tile([128, NT, K], F32, tag="z")
                nc.vector.scalar_tensor_tensor(
                    z[:], Tc[:], 1.0, z2[:], op0=_mult, op1=_add)

                e = sbE.tile([128, NT, K], F32)
                nc.scalar.activation(e[:], z[:],
                                     mybir.ActivationFunctionType.Exp)
                d9 = sbD.tile([128, NT], F32, tag="d")
                nc.vector.tensor_reduce(d9[:], e[:],
                                        axis=mybir.AxisListType.X, op=_add)
                dinv9 = sbD.tile([128, NT], F32, tag="dinv")
                nc.vector.reciprocal(dinv9[:], d9[:])
                awc = sbAw.tile([128, NT, K], BF)
                dinv_bc = dinv9[:].unsqueeze(2).broadcast_to((128, NT, K))
                nc.vector.tensor_mul(awc[:], e[:], dinv_bc)

                if awred == 1 and stage >= 5:
                    awpart = sbD.tile([128, K], F32, tag="ap%d" % (ch % 2))
                    nc.vector.tensor_reduce(
                        awpart[:], awc[:].rearrange("p t k -> p k t"),
                        axis=mybir.AxisListType.X, op=_add)
                    if ch == 0:
                        awacc = awpart
                    else:
                        nxt = sbD.tile([128, K], F32, tag="ac%d" % (ch % 2))
                        nc.vector.tensor_add(nxt[:], awacc[:], awpart[:])
                        awacc = nxt

                for Xp_, xT_ in deferred:
                    nc.scalar.copy(xT_[:], Xp_[:])
                deferred = []

                if stage >= 5:
                    for ti in range(NT):
                        gi = ch * NT + ti
                        pend.append((gi, xTs[ti], awc[:, ti, :]))

            encOut = sbOut.tile([K, C], F32, tag="encOut")
            if stage >= 5:
                for ent in pend:
                    issue_mm2(ent)
                pend = []

                # batch tail: enc = encB + awsum * (-c)
                if awred == 1:
                    nc.tensor.matmul(awsumP[:], awacc[:], onescolf[:],
                                     start=True, stop=True)
                awsum_sb = sbD.tile([K, 1], F32, tag="awsum")
                nc.scalar.copy(awsum_sb[:], awsumP[:])
                nc.vector.scalar_tensor_tensor(
                    encOut[:], cwneg[:], awsum_sb[:], encB[:],
                    op0=_mult, op1=_add)
            else:
                nc.vector.memset(encOut[:], 0.0)
            nc.sync.dma_start(out_d[b], encOut[:])

    nc.finalize()
    return nc


def _prep_inputs(x, codewords, scale):
    xf = np.ascontiguousarray(x.reshape(B, C, N))
    xh = xf.astype(bf16)
    xh4 = xh.reshape(B, CCH, 128, N)

    cw64 = codewords.astype(np.float64)
    sc64 = scale.astype(np.float64)
    alpha = float(sc64.max())
    # Constant softmax offset m ~ alpha * x2: exact softmax is invariant to
    # any per-pixel-constant offset; it only has to keep exp() in range.
    x2flat = np.einsum('bcn,bcn->bn', xf.astype(np.float64), xf.astype(np.float64))
    x2lo, x2hi = float(x2flat.min()), float(x2flat.max())
    m = alpha * 0.5 * (x2lo + x2hi)
    spread = abs(alpha) * 0.5 * (x2hi - x2lo) + 10.0
    assert spread < 60.0, (
        f"constant-offset softmax unsafe: |max_k s - m| can reach {spread:.1f}"
    )
    c2 = (cw64 ** 2).sum(1)
    bias = (sc64 * c2 - m).astype(np.float32)
    w1 = (-2.0 * sc64[:, None] * cw64).astype(bf16)        # (K, C)
    w1t = np.ascontiguousarray(
        w1.T.reshape(CCH, 128, K).transpose(1, 0, 2))       # (128, CCH, K)
    scaleb = np.broadcast_to(scale.astype(np.float32), (128, K)).copy()
    biasb = np.broadcast_to(bias, (128, K)).copy()
    cwneg = np.ascontiguousarray(-codewords.astype(np.float32))  # (K, C)

    # x2 tiled as (B, 128, NTILES): x2p[b, p, g] = x2[b, 128 g + p]
    x2p = np.ascontiguousarray(
        x2flat.astype(np.float32).reshape(B, NTILES, 128).transpose(0, 2, 1))

    consts = {
        "w1t": w1t,
        "biasb": biasb,
        "scaleb": scaleb,
        "cwneg": cwneg,
        "ident": np.eye(128, dtype=bf16),
        "ones_col": np.ones((128, 1), bf16),
        "ones_col_f": np.ones((128, 1), np.float32),
    }
    in_maps = []
    for core in range(NCORES):
        m_ = dict(consts)
        m_["xh"] = xh4[core * BPC:(core + 1) * BPC]
        m_["x2p"] = x2p[core * BPC:(core + 1) * BPC]
        in_maps.append(m_)
    return in_maps


def kernel(x, codewords, scale, _trace=False, _return_results=False, _reps=1):
    key = ("prog", _reps)
    if key not in _compiled:
        _compiled[key] = _build_program(reps=_reps)
    nc = _compiled[key]
    in_maps = _prep_inputs(np.asarray(x), np.asarray(codewords),
                           np.asarray(scale))
    res = run_bass_kernel_spmd(nc, in_maps, list(range(NCORES)), trace=_trace)
    out = np.empty((B, K, C), np.float32)
    for core in range(NCORES):
        o = res.results[core]["enc"]                        # (BPC, K, C)
        for b in range(BPC):
            out[core * BPC + b] = o[b]
    if _return_results:
        return out, res
    return out


# revision 27
# speedup vs baseline: 2.4030x; 1.0782x over previous
"""VQ codebook encoding (nn_Encoding) kernel for 8 Trainium2 NeuronCores.

Reference computation (per batch b):
    xf = x[b].reshape(C, N).T                     # (N, C), N = H*W
    s_nk = scale_k * (||x_n||^2 - 2 x_n.c_k + ||c_k||^2)
    aw = softmax_k(s)
    enc[b] = aw^T xf - (sum_n aw)_k c_k           # (K, C)

Distribution: data-parallel over batch B across the 8 cores (2 batches per
core), codewords/scale replicated.

Final design (per batch, per core):
  - host: xh = bf16(x) (the only big stream, 2B/elem; ~27us/batch DMA at
    the measured 351 GB/s); x2_n = ||x_n||^2 computed exactly on host in
    f64 and streamed as a tiny (128, 72) f32 tile; softmax offset m
    folded into bias (exact softmax is invariant to a per-pixel-constant
    offset, so no online max is needed).
  - mm1 (PE):  T_nk = sum_c xh * W1, W1 = -2*scale_k*c_k (bf16),
               a-tile stationary (load shared with the transpose below);
               accumulated chunk-wide into Tc [128, 9*32] PSUM.
  - transpose: xh tiles transposed on PE (transpose-mode, bf16 PSUM out),
               evacuated to SBUF on DVE (2/3) and ACT (1/3). (DMA XBAR
               transpose measured 185 GB/s and serializes with the HBM
               loads -> PE transpose wins.)
  - softmax (chunk-wide, 9 tiles at once to amortize per-instr overheads
    -- small per-tile DVE/ACT ops cost ~70ns fixed each):
      z = scale_k*x2 + bias_k + Tc   (2 tensor ops + 1 STT on [128,288])
      e = exp(z) on ACT (one [128,288] instr), d = segmented reduce (DVE),
      aw = e * (1/d) bf16 (DVE, broadcast AP)
  - mm2 (PE):  enc[k, c] += sum_n aw * xT   -- aw STATIONARY (32-col load),
               xT moving (512 streams). Issued one chunk behind (pend
               queue) so the PE never waits on the softmax latency.
               awsum is NOT a per-tile PE matmul (that costs ~50us/iter in
               stationary reloads + PSUM bank ping-pong): instead DVE
               reduces awc per chunk + one f32 matmul vs ones at batch end.
  - tail:      enc += awsum_k * (-c_kc) (one STT on 32 partitions), DMA out.
"""

import os

os.environ.setdefault("JAX_PLATFORMS", "")

import numpy as np
import ml_dtypes
from contextlib import ExitStack

import concourse.bacc as bacc
import concourse.bass as bass
import concourse.mybir as mybir
import concourse.tile as tile
from concourse.bass_utils import run_bass_kernel_spmd

bf16 = ml_dtypes.bfloat16
F32 = mybir.dt.float32
BF = mybir.dt.bfloat16

B, C, H, W = 16, 512, 96, 96
N = H * W            # 9216
K = 32
NCORES = 8
BPC = B // NCORES    # batches per core = 2
NCH = 8              # N chunks per batch
NC = N // NCH        # 1152 pixels per chunk
NT = NC // 128       # 9 tiles per chunk
CCH = C // 128       # 4 contraction chunks
NTILES = N // 128    # 72 tiles per batch

_mult = mybir.AluOpType.mult
_add = mybir.AluOpType.add

_compiled = {}


def _build_program(reps=1, stage=5, evac=0, lag=1, awred=1):
    # stage: 0=DMA only, 1=+mm1, 2=+transpose, 3=+evac, 4=+softmax, 5=full
    # evac: 0 = inline [DVE,ACT,DVE]; 1 = all DVE inline;
    #       2 = DVE inline 2/3 + ACT deferred post-softmax 1/3
    # lag: chunks of mm2 backlog kept pending (1 or 2)
    # awred: 0 = per-tile PE awsum matmuls; 1 = DVE chunk reduce + one matmul
    nc = bacc.Bacc("TRN2", target_bir_lowering=False, debug=False,
                   num_devices=NCORES)

    xh_d = nc.dram_tensor("xh", [BPC, CCH, 128, N], BF, kind="ExternalInput").ap()
    x2_d = nc.dram_tensor("x2p", [BPC, 128, NTILES], F32, kind="ExternalInput").ap()
    w1t_d = nc.dram_tensor("w1t", [128, CCH, K], BF, kind="ExternalInput").ap()
    scaleb_d = nc.dram_tensor("scaleb", [128, K], F32, kind="ExternalInput").ap()
    biasb_d = nc.dram_tensor("biasb", [128, K], F32, kind="ExternalInput").ap()
    cwneg_d = nc.dram_tensor("cwneg", [K, C], F32, kind="ExternalInput").ap()
    ident_d = nc.dram_tensor("ident", [128, 128], BF, kind="ExternalInput").ap()
    onescol_d = nc.dram_tensor("ones_col", [128, 1], BF, kind="ExternalInput").ap()
    onescolf_d = nc.dram_tensor("ones_col_f", [128, 1], F32, kind="ExternalInput").ap()
    out_d = nc.dram_tensor("enc", [BPC, K, C], F32, kind="ExternalOutput").ap()

    with tile.TileContext(nc) as tc, ExitStack() as ctx:
        const = ctx.enter_context(tc.tile_pool(name="const", bufs=1))
        xpool = ctx.enter_context(tc.tile_pool(name="xh", bufs=2))
        x2pool = ctx.enter_context(tc.tile_pool(name="x2", bufs=2))
        psT = ctx.enter_context(tc.tile_pool(name="psT", bufs=2, space="PSUM"))
        psX = ctx.enter_context(tc.tile_pool(name="psX", bufs=4, space="PSUM"))
        psE = ctx.enter_context(tc.tile_pool(name="psE", bufs=1, space="PSUM"))
        psA = ctx.enter_context(tc.tile_pool(name="psA", bufs=1, space="PSUM"))
        sbX = ctx.enter_context(tc.tile_pool(name="sbX", bufs=2 * NT + 4))
        sbZ = ctx.enter_context(tc.tile_pool(name="sbZ", bufs=4))
        sbE = ctx.enter_context(tc.tile_pool(name="sbE", bufs=2))
        sbD = ctx.enter_context(tc.tile_pool(name="sbD", bufs=4))
        sbAw = ctx.enter_context(tc.tile_pool(name="sbAw", bufs=4))
        sbOut = ctx.enter_context(tc.tile_pool(name="sbOut", bufs=2))

        w1t = const.tile([128, CCH, K], BF)
        nc.sync.dma_start(w1t[:], w1t_d)
        scaleb = const.tile([128, K], F32)
        nc.sync.dma_start(scaleb[:], scaleb_d)
        biasb = const.tile([128, K], F32)
        nc.sync.dma_start(biasb[:], biasb_d)
        cwneg = const.tile([K, C], F32)
        nc.sync.dma_start(cwneg[:], cwneg_d)
        ident = const.tile([128, 128], BF)
        nc.sync.dma_start(ident[:], ident_d)
        onescol = const.tile([128, 1], BF)
        nc.sync.dma_start(onescol[:], onescol_d)
        onescolf = const.tile([128, 1], F32)
        nc.sync.dma_start(onescolf[:], onescolf_d)

        loop_cm = tc.For_i(0, reps, 1) if reps > 1 else None
        if loop_cm is not None:
            ctx.enter_context(loop_cm)

        for b in range(BPC):
            encB = psE.tile([K, C], F32)       # (k, c) accumulated over N
            awsumP = psA.tile([K, 1], F32)

            x2sb = x2pool.tile([128, NTILES], F32)
            nc.sync.dma_start(x2sb[:], x2_d[b])

            pend = []                          # (gi, xT, aw_slice) for mm2
            deferred = []                      # (Xp, xT) ACT evacs post-softmax

            def issue_mm2(ent):
                gi_, xT_, aw_ = ent
                first = gi_ == 0
                last = gi_ == NTILES - 1
                nc.tensor.matmul(encB[:], aw_, xT_[:],
                                 start=first, stop=last,
                                 skip_group_check=True)
                if awred == 0:
                    nc.tensor.matmul(awsumP[:], aw_, onescol[:],
                                     start=first, stop=last,
                                     skip_group_check=True)

            for ch in range(NCH):
                xh_t = xpool.tile([128, CCH, NC], BF)
                nc.sync.dma_start(
                    xh_t[:],
                    xh_d[b, :, :, ch * NC:(ch + 1) * NC].rearrange("c p n -> p c n"))

                Tc = psT.tile([128, NT, K], F32, tag="T")
                xTs = []
                for ti in range(NT):
                    gi = ch * NT + ti
                    Xp = psX.tile([128, C], BF)
                    for ci in range(CCH):
                        a = xh_t[:, ci, bass.ts(ti, 128)]
                        if stage >= 1:
                            nc.tensor.matmul(Tc[:, ti, :], a, w1t[:, ci, :],
                                             start=(ci == 0), stop=(ci == CCH - 1),
                                             skip_group_check=True)
                        if stage >= 2:
                            nc.tensor.transpose(Xp[:, bass.ts(ci, 128)], a, ident[:])

                    if stage < 3:
                        continue
                    xT = sbX.tile([128, C], BF)
                    if evac == 1 or gi % 3 != 1:
                        nc.vector.tensor_copy(xT[:], Xp[:])
                    elif evac == 0:
                        nc.scalar.copy(xT[:], Xp[:])
                    else:
                        deferred.append((Xp, xT))
                    xTs.append(xT)

                    if stage >= 5 and len(pend) > (lag - 1) * NT:
                        issue_mm2(pend.pop(0))

                if stage < 4:
                    continue
                # chunk-wide softmax over [128, NT, K]
                x2_bc = x2sb[:, ch * NT:(ch + 1) * NT].unsqueeze(2) \
                    .broadcast_to((128, NT, K))
                scale_bc = scaleb[:].unsqueeze(1).broadcast_to((128, NT, K))
                bias_bc = biasb[:].unsqueeze(1).broadcast_to((128, NT, K))

                z1 = sbZ.tile([128, NT, K], F32, tag="z1")
                nc.vector.tensor_mul(z1[:], scale_bc, x2_bc)
                z2 = sbZ.tile([128, NT, K], F32, tag="z2")
                nc.vector.tensor_add(z2[:], z1[:], bias_bc)
                z = sbZ.tile([128, NT, K], F32, tag="z")
                nc.vector.scalar_tensor_tensor(
                    z[:], Tc[:], 1.0, z2[:], op0=_mult, op1=_add)

                e = sbE.tile([128, NT, K], F32)
                nc.scalar.activation(e[:], z[:],
                                     mybir.ActivationFunctionType.Exp)
                d9 = sbD.tile([128, NT], F32, tag="d")
                nc.vector.tensor_reduce(d9[:], e[:],
                                        axis=mybir.AxisListType.X, op=_add)
                dinv9 = sbD.tile([128, NT], F32, tag="dinv")
                nc.vector.reciprocal(dinv9[:], d9[:])
                awc = sbAw.tile([128, NT, K], BF)
                dinv_bc = dinv9[:].unsqueeze(2).broadcast_to((128, NT, K))
                nc.vector.tensor_mul(awc[:], e[:], dinv_bc)

                if awred == 1 and stage >= 5:
                    awpart = sbD.tile([128, K], F32, tag="ap%d" % (ch % 2))
                    nc.vector.tensor_reduce(
                        awpart[:], awc[:].rearrange("p t k -> p k t"),
                        axis=mybir.AxisListType.X, op=_add)
                    if ch == 0:
                        awacc = awpart
                    else:
                        nxt = sbD.tile([128, K], F32, tag="ac%d" % (ch % 2))
                        nc.vector.tensor_add(nxt[:], awacc[:], awpart[:])
                        awacc = nxt

                for Xp_, xT_ in deferred:
                    nc.scalar.copy(xT_[:], Xp_[:])
                deferred = []

                if stage >= 5:
                    for ti in range(NT):
                        gi = ch * NT + ti
                        pend.append((gi, xTs[ti], awc[:, ti, :]))

            encOut = sbOut.tile([K, C], F32, tag="encOut")
            if stage >= 5:
                for ent in pend:
                    issue_mm2(ent)
                pend = []

                # batch tail: enc = encB + awsum * (-c)
                if awred == 1:
                    nc.tensor.matmul(awsumP[:], awacc[:], onescolf[:],
                                     start=True, stop=True)
                awsum_sb = sbD.tile([K, 1], F32, tag="awsum")
                nc.scalar.copy(awsum_sb[:], awsumP[:])
                nc.vector.scalar_tensor_tensor(
                    encOut[:], cwneg[:], awsum_sb[:], encB[:],
                    op0=_mult, op1=_add)
            else:
                nc.vector.memset(encOut[:], 0.0)
            nc.sync.dma_start(out_d[b], encOut[:])

    nc.finalize()
    return nc


def _prep_inputs(x, codewords, scale):
    xf = np.ascontiguousarray(x.reshape(B, C, N))
    xh = xf.astype(bf16)
    xh4 = xh.reshape(B, CCH, 128, N)

    cw64 = codewords.astype(np.float64)
    sc64 = scale.astype(np.float64)
    alpha = float(sc64.max())
    # Constant softmax offset m ~ alpha * x2: exact softmax is invariant to
    # any per-pixel-constant offset; it only has to keep exp() in range.
    x2flat = np.einsum('bcn,bcn->bn', xf.astype(np.float64), xf.astype(np.float64))
    x2lo, x2hi = float(x2flat.min()), float(x2flat.max())
    m = alpha * 0.5 * (x2lo + x2hi)
    spread = abs(alpha) * 0.5 * (x2hi - x2lo) + 10.0
    assert spread < 60.0, (
        f"constant-offset softmax unsafe: |max_k s - m| can reach {spread:.1f}"
    )
    c2 = (cw64 ** 2).sum(1)
    bias = (sc64 * c2 - m).astype(np.float32)
    w1 = (-2.0 * sc64[:, None] * cw64).astype(bf16)        # (K, C)
    w1t = np.ascontiguousarray(
        w1.T.reshape(CCH, 128, K).transpose(1, 0, 2))       # (128, CCH, K)
    scaleb = np.broadcast_to(scale.astype(np.float32), (128, K)).copy()
    biasb = np.broadcast_to(bias, (128, K)).copy()
    cwneg = np.ascontiguousarray(-codewords.astype(np.float32))  # (K, C)

    # x2 tiled as (B, 128, NTILES): x2p[b, p, g] = x2[b, 128 g + p]
    x2p = np.ascontiguousarray(
        x2flat.astype(np.float32).reshape(B, NTILES, 128).transpose(0, 2, 1))

    consts = {
        "w1t": w1t,
        "biasb": biasb,
        "scaleb": scaleb,
        "cwneg": cwneg,
        "ident": np.eye(128, dtype=bf16),
        "ones_col": np.ones((128, 1), bf16),
        "ones_col_f": np.ones((128, 1), np.float32),
    }
    in_maps = []
    for core in range(NCORES):
        m_ = dict(consts)
        m_["xh"] = xh4[core * BPC:(core + 1) * BPC]
        m_["x2p"] = x2p[core * BPC:(core + 1) * BPC]
        in_maps.append(m_)
    return in_maps


def kernel(x, codewords, scale, _trace=False, _return_results=False, _reps=1):
    key = ("prog", _reps)
    if key not in _compiled:
        _compiled[key] = _build_program(reps=_reps)
    nc = _compiled[key]
    in_maps = _prep_inputs(np.asarray(x), np.asarray(codewords),
                           np.asarray(scale))
    res = run_bass_kernel_spmd(nc, in_maps, list(range(NCORES)), trace=_trace)
    out = np.empty((B, K, C), np.float32)
    for core in range(NCORES):
        o = res.results[core]["enc"]                        # (BPC, K, C)
        for b in range(BPC):
            out[core * BPC + b] = o[b]
    if _return_results:
        return out, res
    return out
